# revision 1
# baseline (speedup 1.0000x reference)
"""BandSplit (gather -> per-band MLP -> scatter-add OLA -> /ola) on 8 TRN2 cores.

Strategy
--------
The whole reference computation is linear in x (the per-band pre/post weights,
melbank weights, mask, scatter-add and the final /ola are all linear maps, and
the biases contribute an x-independent constant).  On the host we fold all of
it into a single matrix A of shape (C*F, C*F) mapping the (c, f) spectrum of
one (b, t) token to the (c, f) output spectrum:

    out[b, :, t, :] = A^T @ vec(x[b, :, t, :]) + const

Because every mel band covers a *contiguous* frequency range of width <= Wmax,
A is block-banded: A[(ci, fi), (co, fo)] == 0 unless |fi - fo| < Wmax.  The
device kernel is therefore a banded matmul, data-parallel over the 4096
(b, t) tokens across the 8 NeuronCores (512 tokens/core) with zero
cross-core communication.  The bias constant is folded into a spare padded
row of A (row F, with x padded so column F == 1.0).

Per core: load packed A band tiles (bf16) + its x slice (f32), cast x to bf16
on the vector engine, PE-transpose x tiles to put f on partitions, run banded
bf16 matmuls (fp32 PSUM accumulate), drain to SBUF on the vector engine, DMA
out.  bf16 operands take one PE pass per matmul (fp32 takes two) and enable
the fast weight-load path.
"""

import numpy as np

_P = 128


def _fold_matrix(pre_w, pre_b, post_w, post_b, idx, melw, mask, ola_window):
    """Fold the full reference computation into (A, const).

    A: (C, F, C, F) with out[co, fo] = sum_{ci, fi} x[ci, fi] * A[ci, fi, co, fo]
    const: (C, F) additive constant from the biases.
    """
    K, W = idx.shape
    C = 2
    F = ola_window.shape[0]

    pre_w = np.asarray(pre_w, np.float64)
    post_w = np.asarray(post_w, np.float64)
    pre_b = np.asarray(pre_b, np.float64)
    post_b = np.asarray(post_b, np.float64)
    wts = (np.asarray(melw, np.float64) * np.asarray(mask, np.float64))
    msk = np.asarray(mask, np.float64)
    idx = np.asarray(idx)

    # Per-band folded linear map: M[k, i=(w,cin), j=(w',cout)]
    M = np.einsum('kio,koj->kij', pre_w, post_w).reshape(K, W, C, W, C)
    vals = M * wts[:, :, None, None, None] * msk[:, None, None, :, None]

    fin = idx[:, :, None, None, None].astype(np.int64)
    fout = idx[:, None, None, :, None].astype(np.int64)
    cin = np.arange(C)[None, None, :, None, None]
    cout = np.arange(C)[None, None, None, None, :]
    flat = ((cin * F + fin) * C + cout) * F + fout
    A = np.bincount(
        np.broadcast_to(flat, vals.shape).ravel(), weights=vals.ravel(),
        minlength=C * F * C * F,
    ).reshape(C, F, C, F)
    A /= ola_window[None, None, None, :]

    # Bias constant: (sum_o pre_b[k,o] * post_w[k,o,(w',co)] + post_b) * mask, /ola
    bv = (np.einsum('ko,koj->kj', pre_b, post_w) + post_b).reshape(K, W, C)
    bv = bv * msk[:, :, None]
    cflat = (np.arange(C)[None, None, :] * F + idx[:, :, None]).astype(np.int64)
    const = np.bincount(
        np.broadcast_to(cflat, bv.shape).ravel(), weights=bv.ravel(),
        minlength=C * F,
    ).reshape(C, F)
    const /= ola_window[None, :]
    return A, const


_PROGRAM_CACHE = {}


def _build_program(C, F_PAD, KI, T_CORE, offs, TW, wins, n_cores):
    """Build the Bass/Tile program. Returns the compiled Bass object."""
    import concourse.bass as bass
    import concourse.tile as tile
    import concourse.mybir as mybir
    from concourse import bacc
    from concourse.masks import make_identity

    f32 = mybir.dt.float32
    bf16 = mybir.dt.bfloat16
    P = _P
    TCH = T_CORE // P

    nc = bacc.Bacc("TRN2", target_bir_lowering=False, debug=False,
                   num_devices=n_cores)
    # xs is [TCH, C, P, F_PAD] so each token chunk is one contiguous DMA
    xs = nc.dram_tensor("xs", [TCH, C, P, F_PAD], f32, kind="ExternalInput")
    # ab is packed band windows, laid out [P, TW] (ki, ci, co at offsets offs)
    ab = nc.dram_tensor("ab", [P, TW], bf16, kind="ExternalInput")
    F_OUT = 1025
    # y is written channel-interleaved (col = fo * C + co) in fp16 to halve
    # the store traffic; host de-interleaves and casts back
    f16 = mybir.dt.float16
    y = nc.dram_tensor("y", [TCH, P, F_OUT * C], f16, kind="ExternalOutput")

    # PSUM holds the two output channels interleaved: col = fo * C + co.
    # Windows double; segments split at 512-col (one-bank) boundaries.
    PS_W = 2 * F_PAD
    BANKS = [(b * 512, min(PS_W, (b + 1) * 512)) for b in range((PS_W + 511) // 512)]

    def segments(ki):
        lo, hi = 2 * wins[ki][0], 2 * wins[ki][1]
        segs = []
        for b, (bs, be) in enumerate(BANKS):
            s, e = max(lo, bs), min(hi, be)
            if s < e:
                segs.append((b, s, e))
        return segs

    touches = {}
    for ki in range(KI):
        for ci in range(C):
            for (b, s, e) in segments(ki):
                touches.setdefault(b, []).append((ki, ci, s, e))

    GRP = 3                      # ki per transpose group (one ACT drain each)
    NG = (KI + GRP - 1) // GRP   # groups per channel

    with tile.TileContext(nc) as tc:
        with (
            tc.tile_pool(name="apool", bufs=1) as apool,
            tc.tile_pool(name="xbpool", bufs=4) as xbpool,
            tc.tile_pool(name="xtpool", bufs=3) as xtpool,
            tc.tile_pool(name="opool", bufs=2) as opool,
            tc.tile_pool(name="idpool", bufs=1) as idpool,
            tc.tile_pool(name="pspool", bufs=1, space="PSUM") as pspool,
            tc.tile_pool(name="tpspool", bufs=3, space="PSUM") as tpspool,
        ):
            xbf = {}

            def load_x(tch, split=1):
                # SWDGE cast-during-DMA: f32 DRAM -> bf16 SBUF, one per channel
                for ci in range(C):
                    t = xbpool.tile([P, F_PAD], bf16, tag=f"xbf_{ci}",
                                    name=f"xbf_{tch}_{ci}")
                    step = F_PAD // split
                    for s in range(split):
                        nc.gpsimd.dma_start(
                            t[:, s * step:(s + 1) * step],
                            xs[tch, ci, :, s * step:(s + 1) * step])
                    xbf[(tch, ci)] = t

            ident = idpool.tile([P, P], bf16, name="ident")
            make_identity(nc, ident[:])

            # kick off all x loads up front (xbpool holds every chunk)
            load_x(0, split=3)
            for t_ in range(1, TCH):
                load_x(t_)

            # warm up the PE clock gate (HAM) with throwaway matmuls while the
            # DMAs fill SBUF; output goes to the pt_0 slot which the first real
            # accumulation group overwrites (start=True clears the bank)
            warm = pspool.tile([P, 1024], f32, tag="pt_a", name="warm")
            for _ in range(14):
                nc.tensor.matmul(warm[:, :P], ident[:], ident[:],
                                 start=True, stop=True)

            # A band tiles: one resident SBUF slab, 3 contiguous DMAs
            abig = apool.tile([P, TW], bf16, name="abig")
            bounds = [offs[(k0, 0)] for k0 in range(0, KI, 3)] + [TW]
            for i in range(len(bounds) - 1):
                nc.sync.dma_start(abig[:, bounds[i]:bounds[i + 1]],
                                  ab[:, bounds[i]:bounds[i + 1]])

            def a_tile(ci, ki):
                o = offs[(ki, ci)]
                return abig[:, o:o + 2 * (wins[ki][1] - wins[ki][0])]

            xt = {}

            def transpose_ops(tch, engines=("scalar",)):
                """Thunks, one per (ci, group-of-ki) transpose+drain of chunk tch."""
                for ci in range(C):
                    for g in range(NG):
                        def op(tch=tch, ci=ci, g=g,
                               eng=engines[(ci * NG + g) % len(engines)]):
                            kis = range(g * GRP, min(KI, (g + 1) * GRP))
                            n = len(kis)
                            tps = tpspool.tile([P, GRP * P], bf16, tag="tps",
                                               name=f"tps_{tch}_{ci}_{g}")
                            for j, ki in enumerate(kis):
                                nc.tensor.transpose(
                                    tps[:, j * P:(j + 1) * P],
                                    xbf[(tch, ci)][:, ki * P:(ki + 1) * P], ident[:])
                            xtt = xtpool.tile([P, GRP, P], bf16, tag=f"xt_{ci}_{g}",
                                              name=f"xt_{tch}_{ci}_{g}")
                            if eng == "scalar":
                                nc.scalar.copy(xtt[:, :n], tps[:, :n * P])
                            else:
                                nc.vector.tensor_copy(xtt[:, :n], tps[:, :n * P])
                            xt[(tch, ci, g)] = xtt
                        yield op

            # chunk 0 transposes, alternating drain engines
            t0ops = list(transpose_ops(0, engines=("vector", "scalar")))
            for op in t0ops:
                op()
            # dense warmup burst right before the first real matmuls: ~3.4us of
            # uninterrupted PE activity trips the HAM clock gate to 2.4 GHz so
            # the real matmul stream starts (and stays) at full clock
            for _ in range(30):
                nc.tensor.matmul(warm[:, :P], ident[:], ident[:],
                                 start=True, stop=True)

            for tch in range(TCH):
                nxt = list(transpose_ops(tch + 1)) if tch + 1 < TCH else []

                pt_a = pspool.tile([P, 1024], f32, tag="pt_a", name=f"pta_{tch}")
                pt_b = pspool.tile([P, PS_W - 1024], f32, tag="pt_b",
                                   name=f"ptb_{tch}")

                def pt(s, e):
                    if s < 1024:
                        return pt_a[:, s:e]
                    return pt_b[:, s - 1024:e - 1024]
                # interleave next-chunk transposes into the matmul stream so the
                # PE never idles long enough for HAM to re-throttle
                for ki in range(KI):
                    if ki >= 1 and nxt:
                        nxt.pop(0)()
                    lo2 = 2 * wins[ki][0]
                    for ci in range(C):
                        lhsT = xt[(tch, ci, ki // GRP)][:, ki % GRP, :]
                        for (b, s, e) in segments(ki):
                            order = touches[b]
                            first = order[0] == (ki, ci, s, e)
                            last = order[-1] == (ki, ci, s, e)
                            nc.tensor.matmul(
                                pt(s, e),
                                lhsT,
                                a_tile(ci, ki)[:, s - lo2:e - lo2],
                                start=first, stop=last,
                            )
                for op in nxt:
                    op()

                # drain PSUM -> SBUF -> HBM, contiguous bank-aligned copies
                W_OUT = F_OUT * C
                ot = opool.tile([P, W_OUT], f16, tag="out", name=f"out_{tch}")
                nc.vector.tensor_copy(ot[:, :1024], pt_a[:])
                nc.vector.tensor_copy(ot[:, 1024:W_OUT], pt_b[:, :W_OUT - 1024])
                # ship the first half as soon as its banks finish so the
                # store isn't serialized behind the chunk's last matmul
                nc.sync.dma_start(y[tch, :, :1024], ot[:, :1024])
                nc.sync.dma_start(y[tch, :, 1024:], ot[:, 1024:])

    nc.compile()
    return nc


def kernel(**inputs):
    import ml_dtypes

    x = np.ascontiguousarray(np.asarray(inputs["x"], np.float32))
    B, C, T, F = x.shape
    assert (B, C, F) == (4, 2, 1025), (B, C, F)
    N_CORES = 8
    T_CORE_TOK = B * (T // N_CORES)          # tokens per core

    A, const = _fold_matrix(
        inputs["pre_w"], inputs["pre_b"], inputs["post_w"], inputs["post_b"],
        inputs["idx"], inputs["melw"], inputs["mask"], inputs["ola_window"],
    )

    KI = (F + _P - 1) // _P                   # 9 f-chunks of 128
    F_PAD = KI * _P                           # 1152

    # padded A, with the bias constant folded into spare row F (ci = 0)
    Apad = np.zeros((C, F_PAD, C, F_PAD), np.float32)
    Apad[:, :F, :, :F] = A.astype(np.float32)
    Apad[0, F, :, :F] = const.astype(np.float32)

    # exact nonzero column window per 128-row chunk (same for all channel blocks)
    nz = (Apad != 0).any(axis=(0, 2))          # (F_PAD rows, F_PAD cols)
    wins = []
    for ki in range(KI):
        cols = nz[ki * _P:(ki + 1) * _P].any(axis=0)
        nzc = np.nonzero(cols)[0]
        if len(nzc) == 0:
            lo, hi = ki * _P, ki * _P + 1
        else:
            lo, hi = int(nzc[0]), int(nzc[-1]) + 1
        wins.append((lo, hi))
    # coverage: every output column [0, F) must be written by >= 1 matmul
    covered = np.zeros(F_PAD, bool)
    for lo, hi in wins:
        covered[lo:hi] = True
    assert covered[:F].all(), "window coverage hole"

    # packed band layout: offsets per (ki, ci); the two output channels are
    # interleaved along columns (col = fo * C + co) to match the PSUM layout
    offs = {}
    tw = 0
    for ki in range(KI):
        w2 = (2 * (wins[ki][1] - wins[ki][0]) + 15) // 16 * 16
        for ci in range(C):
            offs[(ki, ci)] = tw
            tw += w2
    TW = tw

    ab = np.zeros((_P, TW), ml_dtypes.bfloat16)
    for ki in range(KI):
        lo, hi = wins[ki]
        for ci in range(C):
            o = offs[(ki, ci)]
            blk = Apad[ci, ki * _P:(ki + 1) * _P, :, lo:hi]       # (P, C, w)
            ab[:, o:o + 2 * (hi - lo)] = blk.transpose(0, 2, 1).reshape(_P, -1)

    key = (C, F_PAD, KI, T_CORE_TOK, TW, tuple(wins), N_CORES)
    if key not in _PROGRAM_CACHE:
        _PROGRAM_CACHE[key] = _build_program(C, F_PAD, KI, T_CORE_TOK, offs, TW, wins, N_CORES)
    nc = _PROGRAM_CACHE[key]

    # shard: core m gets t in [m*T/8, (m+1)*T/8), tokens ordered (b, t_local)
    TS = T // N_CORES
    TCH = T_CORE_TOK // _P
    in_maps = []
    for m in range(N_CORES):
        xs_m = np.zeros((TCH, C, _P, F_PAD), np.float32)
        sl = x[:, :, m * TS:(m + 1) * TS, :]             # (B, C, TS, F)
        tok = sl.transpose(1, 0, 2, 3).reshape(C, T_CORE_TOK, F)
        xs_m[:, :, :, :F] = tok.reshape(C, TCH, _P, F).transpose(1, 0, 2, 3)
        xs_m[:, :, :, F] = 1.0                            # bias row
        in_maps.append({"xs": xs_m, "ab": ab})

    # bass_utils imports antenv.axon_hooks when tracing is requested; this
    # image lacks that module, so provide a no-op stub if it's missing.
    try:
        import antenv.axon_hooks  # noqa: F401
    except ImportError:
        import sys
        import types
        import antenv
        stub = types.ModuleType("antenv.axon_hooks")
        stub.get_axon_ntff_profile_hook = lambda: None
        stub.set_axon_ntff_profile_hook = lambda h: None
        sys.modules["antenv.axon_hooks"] = stub
        antenv.axon_hooks = stub

    from concourse.bass_utils import run_bass_kernel_spmd
    res = run_bass_kernel_spmd(nc, in_maps, core_ids=list(range(N_CORES)))
    globals()["_LAST_RESULT"] = res

    out = np.empty((B, C, T, F), np.float32)
    for m in range(N_CORES):
        # y: (TCH, P, F*C) interleaved; tokens are (b-major, t-minor)
        ym = res.results[m]["y"].astype(np.float32).reshape(TCH, _P, F, C)
        ym = ym.transpose(3, 0, 1, 2).reshape(C, B, TS, F)
        out[:, :, m * TS:(m + 1) * TS, :] = ym.transpose(1, 0, 2, 3)
    return out



# revision 3
# speedup vs baseline: 1.1131x; 1.1131x over previous
"""BandSplit (gather -> per-band MLP -> scatter-add OLA -> /ola) on 8 TRN2 cores.

Strategy
--------
The whole reference computation is linear in x (the per-band pre/post weights,
melbank weights, mask, scatter-add and the final /ola are all linear maps, and
the biases contribute an x-independent constant).  On the host we fold all of
it into a single matrix A of shape (C*F, C*F) mapping the (c, f) spectrum of
one (b, t) token to the (c, f) output spectrum:

    out[b, :, t, :] = A^T @ vec(x[b, :, t, :]) + const

Because every mel band covers a *contiguous* frequency range of width <= Wmax,
A is block-banded: A[(ci, fi), (co, fo)] == 0 unless |fi - fo| < Wmax.  The
device kernel is therefore a banded matmul, data-parallel over the 4096
(b, t) tokens across the 8 NeuronCores (512 tokens/core) with zero
cross-core communication.  The bias constant is folded into a spare padded
row of A (row F, with x padded so column F == 1.0).

v2: the host pre-casts x to bf16 and pre-transposes it into the exact SBUF
layout the matmuls need (f on partitions, tokens on columns), so the device
does zero PE transposes and zero SWDGE cast-DMAs: just 6 big HWDGE loads,
a dense banded-matmul stream into per-bank PSUM tiles, per-bank drains on
the vector/scalar engines (f32 PSUM -> f16 SBUF), and HWDGE stores.
Warmup matmuls on an identity tile trip the HAM clock gate to 2.4 GHz
before the real stream begins.
"""

import numpy as np

_P = 128


def _fold_matrix(pre_w, pre_b, post_w, post_b, idx, melw, mask, ola_window):
    """Fold the full reference computation into (A, const).

    A: (C, F, C, F) with out[co, fo] = sum_{ci, fi} x[ci, fi] * A[ci, fi, co, fo]
    const: (C, F) additive constant from the biases.
    """
    K, W = idx.shape
    C = 2
    F = ola_window.shape[0]

    pre_w = np.asarray(pre_w, np.float64)
    post_w = np.asarray(post_w, np.float64)
    pre_b = np.asarray(pre_b, np.float64)
    post_b = np.asarray(post_b, np.float64)
    wts = (np.asarray(melw, np.float64) * np.asarray(mask, np.float64))
    msk = np.asarray(mask, np.float64)
    idx = np.asarray(idx)

    # Per-band folded linear map: M[k, i=(w,cin), j=(w',cout)]
    M = np.einsum('kio,koj->kij', pre_w, post_w).reshape(K, W, C, W, C)
    vals = M * wts[:, :, None, None, None] * msk[:, None, None, :, None]

    fin = idx[:, :, None, None, None].astype(np.int64)
    fout = idx[:, None, None, :, None].astype(np.int64)
    cin = np.arange(C)[None, None, :, None, None]
    cout = np.arange(C)[None, None, None, None, :]
    flat = ((cin * F + fin) * C + cout) * F + fout
    A = np.bincount(
        np.broadcast_to(flat, vals.shape).ravel(), weights=vals.ravel(),
        minlength=C * F * C * F,
    ).reshape(C, F, C, F)
    A /= ola_window[None, None, None, :]

    # Bias constant: (sum_o pre_b[k,o] * post_w[k,o,(w',co)] + post_b) * mask, /ola
    bv = (np.einsum('ko,koj->kj', pre_b, post_w) + post_b).reshape(K, W, C)
    bv = bv * msk[:, :, None]
    cflat = (np.arange(C)[None, None, :] * F + idx[:, :, None]).astype(np.int64)
    const = np.bincount(
        np.broadcast_to(cflat, bv.shape).ravel(), weights=bv.ravel(),
        minlength=C * F,
    ).reshape(C, F)
    const /= ola_window[None, :]
    return A, const


_PROGRAM_CACHE = {}

_F_OUT = 1025
_C = 2
_KI = 9
_F_PAD = _KI * _P          # 1152
_TCH = 4                   # token chunks (of 128) per core
_PAIRS = 2                 # chunks are loaded in pairs (one DMA per pair)
_PS_W = _C * _F_OUT        # 2050 PSUM output columns (co interleaved: fo*C+co)
_BANKS = [(b * 512, min(_PS_W, (b + 1) * 512)) for b in range((_PS_W + 511) // 512)]


def _build_program(offs, TW, wins, n_cores):
    """Build the Bass/Tile program. Returns the compiled Bass object."""
    import concourse.bass as bass
    import concourse.tile as tile
    import concourse.mybir as mybir
    from concourse import bacc
    from concourse.masks import make_identity

    f32 = mybir.dt.float32
    bf16 = mybir.dt.bfloat16
    f16 = mybir.dt.float16
    P = _P
    KI = _KI
    C = _C
    XCOLS = C * KI * 2 * P        # 4608 cols per pair tile

    nc = bacc.Bacc("TRN2", target_bir_lowering=False, debug=False,
                   num_devices=n_cores)
    # xs: pre-transposed bf16, col = ci*(KI*256) + ki*256 + half*128 + tok
    xs = nc.dram_tensor("xs", [_PAIRS, P, XCOLS], bf16, kind="ExternalInput")
    # ab: packed band windows [P, TW] (ki, ci at offsets offs; co interleaved)
    ab = nc.dram_tensor("ab", [P, TW], bf16, kind="ExternalInput")
    # y: channel-interleaved f16 (col = fo*C + co); host de-interleaves
    y = nc.dram_tensor("y", [_TCH, P, _PS_W], f16, kind="ExternalOutput")

    def segments(ki):
        lo, hi = 2 * wins[ki][0], 2 * wins[ki][1]
        segs = []
        for b, (bs, be) in enumerate(_BANKS):
            s, e = max(lo, bs), min(hi, be)
            if s < e:
                segs.append((b, s, e))
        return segs

    # per-bank (ki, ci, s, e) touch order within one chunk's MM stream
    touches = {}
    for ki in range(KI):
        for ci in range(C):
            for (b, s, e) in segments(ki):
                touches.setdefault(b, []).append((ki, ci, s, e))

    # drain engine per bank: DVE banks 0,2,4 / ACT banks 1,3
    drain_eng = {0: "vector", 1: "scalar", 2: "vector", 3: "scalar", 4: "vector"}

    with tile.TileContext(nc) as tc:
        with (
            tc.tile_pool(name="apool", bufs=1) as apool,
            tc.tile_pool(name="xpool", bufs=1) as xpool,
            tc.tile_pool(name="opool", bufs=2) as opool,
            tc.tile_pool(name="idpool", bufs=1) as idpool,
            tc.tile_pool(name="pspool", bufs=1, space="PSUM") as pspool,
        ):
            ident = idpool.tile([P, P], bf16, name="ident")
            make_identity(nc, ident[:])

            # warmup burst: ~3.5us of PE activity trips the HAM clock gate to
            # 2.4 GHz right as the first x pair lands
            warm = pspool.tile([P, 512], f32, tag="warm", name="warm")
            for _ in range(34):
                nc.tensor.matmul(warm[:, :P], ident[:], ident[:],
                                 start=True, stop=True)

            # loads: ab in 3 ki-triple slabs, xs one DMA per pair, interleaved
            # so the stream's early windows arrive first (SP HWDGE ring, FIFO)
            abig = apool.tile([P, TW], bf16, name="abig")
            bounds = [offs[(k0, 0)] for k0 in range(0, KI, 3)] + [TW]
            xp = [xpool.tile([P, XCOLS], bf16, tag=f"xp{pr}", name=f"xp{pr}")
                  for pr in range(_PAIRS)]
            nc.sync.dma_start(abig[:, bounds[0]:bounds[1]], ab[:, bounds[0]:bounds[1]])
            nc.sync.dma_start(xp[0][:], xs[0])
            nc.sync.dma_start(abig[:, bounds[1]:bounds[2]], ab[:, bounds[1]:bounds[2]])
            nc.sync.dma_start(abig[:, bounds[2]:bounds[3]], ab[:, bounds[2]:bounds[3]])
            nc.sync.dma_start(xp[1][:], xs[1])

            def a_tile(ci, ki):
                o = offs[(ki, ci)]
                return abig[:, o:o + 2 * (wins[ki][1] - wins[ki][0])]

            for tch in range(_TCH):
                pr, half = tch // 2, tch % 2

                bank_t = [
                    pspool.tile([P, be - bs], f32, tag=f"bk{b}",
                                name=f"bk{b}_{tch}")
                    for b, (bs, be) in enumerate(_BANKS)
                ]

                for ki in range(KI):
                    lo2 = 2 * wins[ki][0]
                    for ci in range(C):
                        o = ci * (KI * 2 * P) + ki * 2 * P + half * P
                        lhsT = xp[pr][:, o:o + P]
                        for (b, s, e) in segments(ki):
                            order = touches[b]
                            first = order[0] == (ki, ci, s, e)
                            last = order[-1] == (ki, ci, s, e)
                            bs = _BANKS[b][0]
                            nc.tensor.matmul(
                                bank_t[b][:, s - bs:e - bs],
                                lhsT,
                                a_tile(ci, ki)[:, s - lo2:e - lo2],
                                start=first, stop=last,
                            )

                # per-bank drains (f32 PSUM -> f16 SBUF) split across DVE/ACT,
                # then ship each half as soon as its banks are drained
                ot = opool.tile([P, _PS_W], f16, tag="out", name=f"out_{tch}")
                for b, (bs, be) in enumerate(_BANKS):
                    if drain_eng[b] == "vector":
                        nc.vector.tensor_copy(ot[:, bs:be], bank_t[b][:])
                    else:
                        nc.scalar.copy(ot[:, bs:be], bank_t[b][:])
                    if be == 1024:
                        nc.scalar.dma_start(y[tch, :, :1024], ot[:, :1024])
                nc.scalar.dma_start(y[tch, :, 1024:], ot[:, 1024:])

    nc.compile()
    return nc


def kernel(**inputs):
    import ml_dtypes

    x = np.ascontiguousarray(np.asarray(inputs["x"], np.float32))
    B, C, T, F = x.shape
    assert (B, C, F) == (4, 2, 1025), (B, C, F)
    N_CORES = 8
    TS = T // N_CORES                        # 128 frames per core

    A, const = _fold_matrix(
        inputs["pre_w"], inputs["pre_b"], inputs["post_w"], inputs["post_b"],
        inputs["idx"], inputs["melw"], inputs["mask"], inputs["ola_window"],
    )

    # padded A, with the bias constant folded into spare row F (ci = 0)
    Apad = np.zeros((C, _F_PAD, C, _F_PAD), np.float32)
    Apad[:, :F, :, :F] = A.astype(np.float32)
    Apad[0, F, :, :F] = const.astype(np.float32)

    # exact nonzero column window per 128-row chunk (same for all channel blocks)
    nz = (Apad != 0).any(axis=(0, 2))          # (F_PAD rows, F_PAD cols)
    wins = []
    for ki in range(_KI):
        cols = nz[ki * _P:(ki + 1) * _P].any(axis=0)
        nzc = np.nonzero(cols)[0]
        if len(nzc) == 0:
            lo, hi = ki * _P, ki * _P + 1
        else:
            lo, hi = int(nzc[0]), int(nzc[-1]) + 1
        wins.append((lo, min(hi, F)))
    # coverage: every output column [0, F) must be written by >= 1 matmul
    covered = np.zeros(_F_PAD, bool)
    for lo, hi in wins:
        covered[lo:hi] = True
    assert covered[:F].all(), "window coverage hole"

    # packed band layout: offsets per (ki, ci); the two output channels are
    # interleaved along columns (col = fo * C + co) to match the PSUM layout
    offs = {}
    tw = 0
    for ki in range(_KI):
        w2 = (2 * (wins[ki][1] - wins[ki][0]) + 15) // 16 * 16
        for ci in range(C):
            offs[(ki, ci)] = tw
            tw += w2
    TW = tw

    ab = np.zeros((_P, TW), ml_dtypes.bfloat16)
    for ki in range(_KI):
        lo, hi = wins[ki]
        for ci in range(C):
            o = offs[(ki, ci)]
            blk = Apad[ci, ki * _P:(ki + 1) * _P, :, lo:hi]       # (P, C, w)
            ab[:, o:o + 2 * (hi - lo)] = blk.transpose(0, 2, 1).reshape(_P, -1)

    key = (TW, tuple(wins), N_CORES)
    if key not in _PROGRAM_CACHE:
        _PROGRAM_CACHE[key] = _build_program(offs, TW, wins, N_CORES)
    nc = _PROGRAM_CACHE[key]

    # host-side pad + bias column + bf16 cast + transpose into device layout:
    # xs[pr, p, ci*2304 + ki*256 + half*128 + t] = xpad[2pr+half, ci, mTS+t, 128ki+p]
    xpad = np.zeros((B, C, T, _F_PAD), np.float32)
    xpad[:, :, :, :F] = x
    xpad[:, :, :, F] = 1.0                            # bias row
    xpad = xpad.astype(ml_dtypes.bfloat16)

    in_maps = []
    for m in range(N_CORES):
        sl = xpad[:, :, m * TS:(m + 1) * TS, :]       # (B, C, 128, F_PAD)
        sl = sl.reshape(_PAIRS, 2, C, TS, _KI, _P)    # (pr, half, ci, t, ki, p)
        xs_m = np.ascontiguousarray(
            sl.transpose(0, 5, 2, 4, 1, 3)            # (pr, p, ci, ki, half, t)
        ).reshape(_PAIRS, _P, C * _KI * 2 * TS)
        in_maps.append({"xs": xs_m, "ab": ab})

    # bass_utils imports antenv.axon_hooks when tracing is requested; this
    # image lacks that module, so provide a no-op stub if it's missing.
    try:
        import antenv.axon_hooks  # noqa: F401
    except ImportError:
        import sys
        import types
        import antenv
        stub = types.ModuleType("antenv.axon_hooks")
        stub.get_axon_ntff_profile_hook = lambda: None
        stub.set_axon_ntff_profile_hook = lambda h: None
        sys.modules["antenv.axon_hooks"] = stub
        antenv.axon_hooks = stub

    from concourse.bass_utils import run_bass_kernel_spmd
    res = run_bass_kernel_spmd(nc, in_maps, core_ids=list(range(N_CORES)))
    globals()["_LAST_RESULT"] = res

    out = np.empty((B, C, T, F), np.float32)
    for m in range(N_CORES):
        # y: (TCH, P, F*C) interleaved; chunk tch == batch tch of this t-slice
        ym = res.results[m]["y"].astype(np.float32).reshape(_TCH, _P, F, C)
        ym = ym.transpose(0, 3, 1, 2)                 # (b, c, t, f)
        out[:, :, m * TS:(m + 1) * TS, :] = ym
    return out


# revision 4
# speedup vs baseline: 1.3382x; 1.2022x over previous
"""BandSplit (gather -> per-band MLP -> scatter-add OLA -> /ola) on 8 TRN2 cores.

Strategy
--------
The whole reference computation is linear in x (the per-band pre/post weights,
melbank weights, mask, scatter-add and the final /ola are all linear maps, and
the biases contribute an x-independent constant).  On the host we fold all of
it into a single matrix A of shape (C*F, C*F) mapping the (c, f) spectrum of
one (b, t) token to the (c, f) output spectrum:

    out[b, :, t, :] = A^T @ vec(x[b, :, t, :]) + const

Because every mel band covers a *contiguous* frequency range of width <= Wmax,
A is block-banded: A[(ci, fi), (co, fo)] == 0 unless |fi - fo| < Wmax.  The
device kernel is therefore a banded matmul, data-parallel over the 4096
(b, t) tokens across the 8 NeuronCores (512 tokens/core) with zero
cross-core communication.  The bias constant is folded into a spare padded
row of A (row F, with x padded so column F == 1.0).

v3: the host pre-casts x to bf16 and pre-transposes it into the exact SBUF
layout the matmuls need (f on partitions, tokens on columns, ki-major so
early frequency chunks arrive first), so the device does zero PE transposes
and zero SWDGE cast-DMAs.  Loads are issued from the scalar (ACT) sequencer,
which boots ~1.7us earlier than sync; slabs are ordered so the matmul
stream's first windows land first.  Warmup matmuls fill the engine-boot ->
first-data window to trip the HAM clock gate to full clock.  Matmuls
accumulate into per-bank PSUM tiles; drains (f32 PSUM -> f16 SBUF) are
split across the vector/scalar engines and stores go out on the sync ring.
"""

import numpy as np

_P = 128


def _fold_matrix(pre_w, pre_b, post_w, post_b, idx, melw, mask, ola_window):
    """Fold the full reference computation into (A, const).

    A: (C, F, C, F) with out[co, fo] = sum_{ci, fi} x[ci, fi] * A[ci, fi, co, fo]
    const: (C, F) additive constant from the biases.
    """
    K, W = idx.shape
    C = 2
    F = ola_window.shape[0]

    pre_w = np.asarray(pre_w, np.float64)
    post_w = np.asarray(post_w, np.float64)
    pre_b = np.asarray(pre_b, np.float64)
    post_b = np.asarray(post_b, np.float64)
    wts = (np.asarray(melw, np.float64) * np.asarray(mask, np.float64))
    msk = np.asarray(mask, np.float64)
    idx = np.asarray(idx)

    # Per-band folded linear map: M[k, i=(w,cin), j=(w',cout)]
    M = np.einsum('kio,koj->kij', pre_w, post_w).reshape(K, W, C, W, C)
    vals = M * wts[:, :, None, None, None] * msk[:, None, None, :, None]

    fin = idx[:, :, None, None, None].astype(np.int64)
    fout = idx[:, None, None, :, None].astype(np.int64)
    cin = np.arange(C)[None, None, :, None, None]
    cout = np.arange(C)[None, None, None, None, :]
    flat = ((cin * F + fin) * C + cout) * F + fout
    A = np.bincount(
        np.broadcast_to(flat, vals.shape).ravel(), weights=vals.ravel(),
        minlength=C * F * C * F,
    ).reshape(C, F, C, F)
    A /= ola_window[None, None, None, :]

    # Bias constant: (sum_o pre_b[k,o] * post_w[k,o,(w',co)] + post_b) * mask, /ola
    bv = (np.einsum('ko,koj->kj', pre_b, post_w) + post_b).reshape(K, W, C)
    bv = bv * msk[:, :, None]
    cflat = (np.arange(C)[None, None, :] * F + idx[:, :, None]).astype(np.int64)
    const = np.bincount(
        np.broadcast_to(cflat, bv.shape).ravel(), weights=bv.ravel(),
        minlength=C * F,
    ).reshape(C, F)
    const /= ola_window[None, :]
    return A, const


_PROGRAM_CACHE = {}

_F_OUT = 1025
_C = 2
_KI = 9
_F_PAD = _KI * _P          # 1152
_TCH = 4                   # token chunks (of 128) per core
_PAIRS = 2                 # chunks are loaded in pairs
_PS_W = _C * _F_OUT        # 2050 PSUM output columns (co interleaved: fo*C+co)
_BANKS = [(b * 512, min(_PS_W, (b + 1) * 512)) for b in range((_PS_W + 511) // 512)]
_KCOL = _C * 2 * _P        # 512 x-cols per ki block (ci, half, tok)


def _build_program(offs, TW, wins, n_cores):
    """Build the Bass/Tile program. Returns the compiled Bass object."""
    import concourse.bass as bass
    import concourse.tile as tile
    import concourse.mybir as mybir
    from concourse import bacc
    from concourse.masks import make_identity

    f32 = mybir.dt.float32
    bf16 = mybir.dt.bfloat16
    f16 = mybir.dt.float16
    P = _P
    KI = _KI
    C = _C
    XCOLS = KI * _KCOL            # 4608 cols per pair tile (ki-major)

    nc = bacc.Bacc("TRN2", target_bir_lowering=False, debug=False,
                   num_devices=n_cores)
    # xs: pre-transposed bf16, col = ki*512 + ci*256 + half*128 + tok
    xs = nc.dram_tensor("xs", [_PAIRS, P, XCOLS], bf16, kind="ExternalInput")
    # ab: packed band windows [P, TW] (ki, ci at offsets offs; co interleaved)
    ab = nc.dram_tensor("ab", [P, TW], bf16, kind="ExternalInput")
    # y: channel-interleaved f16 (col = fo*C + co); host de-interleaves
    y = nc.dram_tensor("y", [_TCH, P, _PS_W], f16, kind="ExternalOutput")

    def segments(ki):
        lo, hi = 2 * wins[ki][0], 2 * wins[ki][1]
        segs = []
        for b, (bs, be) in enumerate(_BANKS):
            s, e = max(lo, bs), min(hi, be)
            if s < e:
                segs.append((b, s, e))
        return segs

    # per-bank (ki, ci, s, e) touch order within one chunk's MM stream
    touches = {}
    for ki in range(KI):
        for ci in range(C):
            for (b, s, e) in segments(ki):
                touches.setdefault(b, []).append((ki, ci, s, e))

    # drain engine per bank: DVE banks 0,2,4 / ACT banks 1,3
    drain_eng = {0: "vector", 1: "scalar", 2: "vector", 3: "scalar", 4: "vector"}

    with tile.TileContext(nc) as tc:
        with (
            tc.tile_pool(name="apool", bufs=1) as apool,
            tc.tile_pool(name="xpool", bufs=1) as xpool,
            tc.tile_pool(name="opool", bufs=2) as opool,
            tc.tile_pool(name="idpool", bufs=1) as idpool,
            tc.tile_pool(name="pspool", bufs=1, space="PSUM") as pspool,
        ):
            ident = idpool.tile([P, P], bf16, name="ident")
            make_identity(nc, ident[:])

            # loads on the ACT (scalar) HWDGE ring -- ACT boots earlier than
            # SP, and the SP ring is kept for stores so they never queue
            # behind loads.  Slab order matches the stream's data needs.
            abig = apool.tile([P, TW], bf16, name="abig")
            bounds = [offs[(k0, 0)] for k0 in range(0, KI, 3)] + [TW]
            xp = [xpool.tile([P, XCOLS], bf16, tag=f"xp{pr}", name=f"xp{pr}")
                  for pr in range(_PAIRS)]
            XS = 3 * _KCOL            # x slab: one ki-triple (1536 cols)
            nc.scalar.dma_start(abig[:, bounds[0]:bounds[1]],
                                ab[:, bounds[0]:bounds[1]])
            nc.scalar.dma_start(xp[0][:, 0:XS], xs[0, :, 0:XS])
            nc.scalar.dma_start(abig[:, bounds[1]:bounds[2]],
                                ab[:, bounds[1]:bounds[2]])
            nc.scalar.dma_start(xp[0][:, XS:2 * XS], xs[0, :, XS:2 * XS])
            nc.scalar.dma_start(abig[:, bounds[2]:bounds[3]],
                                ab[:, bounds[2]:bounds[3]])
            nc.scalar.dma_start(xp[0][:, 2 * XS:], xs[0, :, 2 * XS:])
            nc.scalar.dma_start(xp[1][:], xs[1])

            # warmup burst: PE boots ~1-3us before the first slabs land; use
            # that window to trip the HAM clock gate so the real stream starts
            # at full clock
            warm = pspool.tile([P, 512], f32, tag="warm", name="warm")
            for _ in range(40):
                nc.tensor.matmul(warm[:, :P], ident[:], ident[:],
                                 start=True, stop=True)

            def a_tile(ci, ki):
                o = offs[(ki, ci)]
                return abig[:, o:o + 2 * (wins[ki][1] - wins[ki][0])]

            for tch in range(_TCH):
                pr, half = tch // 2, tch % 2

                bank_t = [
                    pspool.tile([P, be - bs], f32, tag=f"bk{b}",
                                name=f"bk{b}_{tch}")
                    for b, (bs, be) in enumerate(_BANKS)
                ]

                for ki in range(KI):
                    lo2 = 2 * wins[ki][0]
                    for ci in range(C):
                        o = ki * _KCOL + ci * 2 * P + half * P
                        lhsT = xp[pr][:, o:o + P]
                        for (b, s, e) in segments(ki):
                            order = touches[b]
                            first = order[0] == (ki, ci, s, e)
                            last = order[-1] == (ki, ci, s, e)
                            bs = _BANKS[b][0]
                            nc.tensor.matmul(
                                bank_t[b][:, s - bs:e - bs],
                                lhsT,
                                a_tile(ci, ki)[:, s - lo2:e - lo2],
                                start=first, stop=last,
                            )

                # per-bank drains (f32 PSUM -> f16 SBUF) split across DVE/ACT,
                # then ship each half as soon as its banks are drained
                ot = opool.tile([P, _PS_W], f16, tag="out", name=f"out_{tch}")
                for b, (bs, be) in enumerate(_BANKS):
                    if drain_eng[b] == "vector":
                        nc.vector.tensor_copy(ot[:, bs:be], bank_t[b][:])
                    else:
                        nc.scalar.copy(ot[:, bs:be], bank_t[b][:])
                    if be == 1024:
                        nc.sync.dma_start(y[tch, :, :1024], ot[:, :1024])
                nc.sync.dma_start(y[tch, :, 1024:], ot[:, 1024:])

    nc.compile()
    return nc


def kernel(**inputs):
    import ml_dtypes

    x = np.ascontiguousarray(np.asarray(inputs["x"], np.float32))
    B, C, T, F = x.shape
    assert (B, C, F) == (4, 2, 1025), (B, C, F)
    N_CORES = 8
    TS = T // N_CORES                        # 128 frames per core

    A, const = _fold_matrix(
        inputs["pre_w"], inputs["pre_b"], inputs["post_w"], inputs["post_b"],
        inputs["idx"], inputs["melw"], inputs["mask"], inputs["ola_window"],
    )

    # padded A, with the bias constant folded into spare row F (ci = 0)
    Apad = np.zeros((C, _F_PAD, C, _F_PAD), np.float32)
    Apad[:, :F, :, :F] = A.astype(np.float32)
    Apad[0, F, :, :F] = const.astype(np.float32)

    # nonzero column window per 128-row chunk; trim edge columns whose entire
    # column magnitude is tiny (mel-band tails) -- bounded output error well
    # below the bf16 noise floor, saves matmul columns
    colmax = np.abs(Apad).max(axis=(0, 2))     # (F_PAD rows, F_PAD cols)
    TRIM = 2e-4 * colmax.max()
    wins = []
    for ki in range(_KI):
        cm = colmax[ki * _P:(ki + 1) * _P].max(axis=0)
        nzc = np.nonzero(cm > TRIM)[0]
        if len(nzc) == 0:
            lo, hi = ki * _P, ki * _P + 1
        else:
            lo, hi = int(nzc[0]), int(nzc[-1]) + 1
        wins.append((lo, min(hi, F)))
    # coverage: every output column [0, F) must be written by >= 1 matmul
    covered = np.zeros(_F_PAD, bool)
    for lo, hi in wins:
        covered[lo:hi] = True
    assert covered[:F].all(), "window coverage hole"

    # packed band layout: offsets per (ki, ci); the two output channels are
    # interleaved along columns (col = fo * C + co) to match the PSUM layout
    offs = {}
    tw = 0
    for ki in range(_KI):
        w2 = (2 * (wins[ki][1] - wins[ki][0]) + 15) // 16 * 16
        for ci in range(C):
            offs[(ki, ci)] = tw
            tw += w2
    TW = tw

    ab = np.zeros((_P, TW), ml_dtypes.bfloat16)
    for ki in range(_KI):
        lo, hi = wins[ki]
        for ci in range(C):
            o = offs[(ki, ci)]
            blk = Apad[ci, ki * _P:(ki + 1) * _P, :, lo:hi]       # (P, C, w)
            ab[:, o:o + 2 * (hi - lo)] = blk.transpose(0, 2, 1).reshape(_P, -1)

    key = (TW, tuple(wins), N_CORES)
    if key not in _PROGRAM_CACHE:
        _PROGRAM_CACHE[key] = _build_program(offs, TW, wins, N_CORES)
    nc = _PROGRAM_CACHE[key]

    # host-side pad + bias column + bf16 cast + transpose into device layout:
    # xs[pr, p, ki*512 + ci*256 + half*128 + t] = xpad[2pr+half, ci, mTS+t, 128ki+p]
    xpad = np.zeros((B, C, T, _F_PAD), np.float32)
    xpad[:, :, :, :F] = x
    xpad[:, :, :, F] = 1.0                            # bias row
    xpad = xpad.astype(ml_dtypes.bfloat16)

    in_maps = []
    for m in range(N_CORES):
        sl = xpad[:, :, m * TS:(m + 1) * TS, :]       # (B, C, 128, F_PAD)
        sl = sl.reshape(_PAIRS, 2, C, TS, _KI, _P)    # (pr, half, ci, t, ki, p)
        xs_m = np.ascontiguousarray(
            sl.transpose(0, 5, 4, 2, 1, 3)            # (pr, p, ki, ci, half, t)
        ).reshape(_PAIRS, _P, _KI * _KCOL)
        in_maps.append({"xs": xs_m, "ab": ab})

    # bass_utils imports antenv.axon_hooks when tracing is requested; this
    # image lacks that module, so provide a no-op stub if it's missing.
    try:
        import antenv.axon_hooks  # noqa: F401
    except ImportError:
        import sys
        import types
        import antenv
        stub = types.ModuleType("antenv.axon_hooks")
        stub.get_axon_ntff_profile_hook = lambda: None
        stub.set_axon_ntff_profile_hook = lambda h: None
        sys.modules["antenv.axon_hooks"] = stub
        antenv.axon_hooks = stub

    from concourse.bass_utils import run_bass_kernel_spmd
    res = run_bass_kernel_spmd(nc, in_maps, core_ids=list(range(N_CORES)))
    globals()["_LAST_RESULT"] = res

    out = np.empty((B, C, T, F), np.float32)
    for m in range(N_CORES):
        # y: (TCH, P, F*C) interleaved; chunk tch == batch tch of this t-slice
        ym = res.results[m]["y"].astype(np.float32).reshape(_TCH, _P, F, C)
        ym = ym.transpose(0, 3, 1, 2)                 # (b, c, t, f)
        out[:, :, m * TS:(m + 1) * TS, :] = ym
    return out
